# revision 54
# baseline (speedup 1.0000x reference)
"""DMR induction routing kernel for Trainium2 (Bass/Tile), 8-core data-parallel.

Problem: nn_DMRInduction. Full inputs:
  m [128, 768], q [256, 768], W [768, 765], b [765] -> out [256, 765] fp32.

Sharding: Q=256 split 8 ways (32 queries/core); m, W, b replicated.

v2 layout/dataflow (per core):
  - hat_m_r  [I=128, 1024] (I on partitions; cols 0..764 = m @ W, cols 765+ zero
      so the final per-class matmuls can stream N=256 on the fp32r fast path)
  - mTc1 [128, C, 128] / mTc2 [34, C, 128]: per-class transposes of hat_m
      (d on partitions). mTc2 row 32 = -mean_d(hat_m) per (c,i), computed by a
      ones-matmul over mTc + scaled copy, so the pearson numerator matmul is
      centered for free (row 32 of tqB carries colsum(tq)).
  - tq [d, (c,q)]: computed DIRECTLY transposed from qT/W with 60 small
      matmuls into PSUM (no PE eye-transposes for the q side at all).
  - routing state a, p, dsp: [I=128, C*Q=160].
  - squash/pearson scalars on [1, 160] rows; broadcasts via 1-row matmuls.
  - iteration v is consumed straight from PSUM (svA = hv_psum * 0.5s); the
    m-dot-v matmul runs on the scaled v, so a += p*(2*mdv') needs one fused op.
"""
import os
import sys

for _p in ("/opt/trn_rl_repo", "/root/.axon_site/_ro/trn_rl_repo"):
    if os.path.isdir(_p) and _p not in sys.path:
        sys.path.insert(0, _p)

import numpy as np
import concourse.bass as bass
import concourse.bacc as bacc
import concourse.mybir as mybir
import concourse.tile as tile
from concourse.bass_utils import run_bass_kernel_spmd

F32 = mybir.dt.float32
# float32r uses the fast PE path (1 cyc/row at N>=256 vs 4) at ~2.5e-4
# scale-relative output error (tolerance is 2e-2). KERNEL_MM_DT=float32
# restores exact matmuls.
DT = getattr(mybir.dt, os.environ.get("KERNEL_MM_DT", "float32r"))
# input-operand dtype (W, mT, qT): bf16 halves HBM traffic and runs the PE at
# 1 cyc/row for any moving width; costs ~2e-3 relative error (gate is 2e-2).
WDT = getattr(mybir.dt, os.environ.get("KERNEL_W_DT", "bfloat16"))
# routing-state (matmul moving-operand) dtype
RDT = getattr(mybir.dt, os.environ.get("KERNEL_R_DT", "bfloat16"))

NCORES = 8
I = 128         # memory capsules
C = 5           # capsule classes
D = 153         # dim per capsule
CD = C * D      # 765
K = 768         # input dim
KC = K // 128   # 6 contraction chunks
QL = 32         # queries per core
CQ = C * QL     # 160
NPAD = 768      # W padded to 768 cols so fp32r matmuls stream N>=256
HM_W = 1024     # hat_m_r padded width (final matmuls read 256-wide windows)
EPS = 1e-8
AX = mybir.AxisListType.X
MUL = mybir.AluOpType.mult
ADD = mybir.AluOpType.add
SUB = mybir.AluOpType.subtract
ACT = mybir.ActivationFunctionType


def build(with_bias: bool, dbg: bool = False):
    nc = bacc.Bacc("TRN2", target_bir_lowering=False, debug=False)

    # qm packs qT [128, 6*32] and mT [128, 6*128] k-major so each partition row
    # is one contiguous run (>=512B => no 2x DMA latency multiplier) and both
    # load with a single dma_start (HWDGE descriptor gen is 625ns each,
    # serialized on one queue). W is k-major [128, 6*768], loaded in 3 k-pieces
    # so projection matmuls start before the full W lands.
    qm_d = nc.dram_tensor("qm", [128, KC * (QL + I)], WDT, kind="ExternalInput")
    W_d = nc.dram_tensor("Wp", [128, KC * NPAD], WDT, kind="ExternalInput")
    if with_bias:
        b_d = nc.dram_tensor("b", [1, CD], F32, kind="ExternalInput")
    out_d = nc.dram_tensor("out", [QL, CD], F32, kind="ExternalOutput")
    dbg_d = {}
    if dbg:
        for nm, shp in [("hatm", [128, CD]),
                        ("p1", [128, CQ]), ("a1", [128, CQ]),
                        ("p2", [128, CQ]), ("a2", [128, CQ]), ("p3", [128, CQ]),
                        ("yn21", [1, CQ]), ("yn22", [1, CQ]), ("yn23", [1, CQ]),
                        ("n2qd", [QL, C])]:
            dbg_d[nm] = nc.dram_tensor("dbg_" + nm, shp, F32, kind="ExternalOutput")
        for nm, shp in [("tqA0", [128, CQ]), ("tqB0", [34, CQ]),
                        ("mTc1d", [128, C * 128]), ("mTc2d", [34, C * 128])]:
            dbg_d[nm] = nc.dram_tensor("dbg_" + nm, shp, RDT, kind="ExternalOutput")

    with tile.TileContext(nc) as tc:
        with (
            tc.tile_pool(name="sb", bufs=1) as sb,
            tc.tile_pool(name="sb2", bufs=3) as sb2,
        ):
            # ---------------- input DMAs (order = HWDGE serial order) -------
            qm_sb = sb.tile([128, KC * (QL + I)], WDT, tag="qm")
            W_sb = sb.tile([128, KC, NPAD], WDT, tag="W")
            nc.sync.dma_start(qm_sb[:], qm_d[:])
            Wr = W_d[:].rearrange("p (k n) -> p k n", k=KC)
            for piece in range(3):
                nc.sync.dma_start(W_sb[:, 2 * piece:2 * piece + 2, :],
                                  Wr[:, 2 * piece:2 * piece + 2, :])
            if with_bias:
                b_sb = sb.tile([1, CD], F32, tag="b")
                nc.sync.dma_start(b_sb[:], b_d[:])
            qT_sb = qm_sb[:, 0:KC * QL].rearrange("p (k n) -> p k n", k=KC)
            mT_sb = qm_sb[:, KC * QL:].rearrange("p (k n) -> p k n", k=KC)

            # ---------------- constants (no DMA) ----------------------------
            # float32r tiles cannot be memset directly; memset F32 staging and
            # copy through Act/DVE (engine writes perform the f32r rounding).
            zf = sb.tile([128, 640], F32, tag="zf")
            nc.vector.memset(zf[:], 0.0)
            of = sb.tile([128, 1], F32, tag="of")
            nc.vector.memset(of[:], 1.0)
            o1f = sb.tile([1, 128], F32, tag="o1f")
            nc.vector.memset(o1f[:], 1.0)
            nhf = sb.tile([1, 128], F32, tag="nhf")
            nc.vector.memset(nhf[:], -0.5)
            epsb = sb.tile([128, 1], F32, tag="epsb")
            nc.vector.memset(epsb[:], EPS)
            ln2b = sb.tile([1, 1], F32, tag="ln2b")
            nc.vector.memset(ln2b[:], 0.6931471805599453)
            # identity for PE transposes, built on-chip (a DMA would arrive
            # ~12us in, after the W pieces, and gate the whole transpose block)
            eye = sb.tile([128, 128], F32, tag="eye")
            nc.vector.memset(eye[:], 1.0)
            nc.gpsimd.affine_select(eye[:], eye[:], pattern=[[-1, 128]],
                                    compare_op=mybir.AluOpType.is_equal,
                                    fill=0.0, base=0, channel_multiplier=1)
            eyeb = sb.tile([128, 128], RDT, tag="eyeb")
            nc.gpsimd.tensor_copy(eyeb[:], eye[:])

            ones1 = sb.tile([1, 128], RDT, tag="ones1")
            nc.scalar.copy(ones1[:], o1f[:])
            mones1 = sb.tile([1, 128], RDT, tag="mones1")
            nc.scalar.activation(mones1[:], o1f[:], ACT.Copy, scale=-1.0)
            onesF = sb.tile([128, 1], RDT, tag="onesF")
            nc.scalar.copy(onesF[:], of[:])
            if with_bias:
                onesq = sb.tile([1, QL], RDT, tag="onesq")
                nc.vector.tensor_copy(onesq[:], o1f[:, 0:QL])

            # persistent tiles that need zero rows
            hat_m_bf = sb.tile([128, HM_W], RDT, tag="hatmbf")
            nc.vector.memset(hat_m_bf[:, CD:HM_W], 0.0)
            mTc1 = sb.tile([128, C, 128], RDT, tag="mTc1")
            mTc2 = sb.tile([34, C, 128], RDT, tag="mTc2")
            nc.scalar.copy(mTc2[:].rearrange("p c n -> p (c n)"), zf[0:34, 0:640])
            # moving-operand stand-in for softmax(0) = 1/C at iteration 0
            c02f = sb.tile([128, QL], F32, tag="c02f")
            nc.vector.memset(c02f[:], 1.0 / C)
            c02_t = sb.tile([128, QL], RDT, tag="c02")
            nc.scalar.copy(c02_t[:], c02f[:])
            tqA = sb.tile([128, C, QL], RDT, tag="tqA")
            tqB = sb.tile([34, C, QL], RDT, tag="tqB")
            nc.vector.tensor_copy(tqB[:].rearrange("p c q -> p (c q)"), zf[0:34, 0:CQ])
            # vA / vB34 hold hv staged to SBUF; vB34 rows 25-31,33 stay zero and
            # row 32 carries colsum(hv) so the centered numv matmul reads one tile
            vA = sb.tile([128, C, QL], RDT, tag="vA")
            vB34 = sb.tile([34, C, QL], RDT, tag="vB34")
            nc.scalar.copy(vB34[:].rearrange("p c q -> p (c q)"), zf[0:34, 0:CQ])

            tqAf = tqA[:].rearrange("p c q -> p (c q)")
            tqBf25 = tqB[0:25].rearrange("p c q -> p (c q)")
            tqB32f = tqB[32:33, :, :].rearrange("p c q -> p (c q)")
            vAf = vA[:].rearrange("p c q -> p (c q)")
            vBf25 = vB34[0:25].rearrange("p c q -> p (c q)")
            vB32f = vB34[32:33, :, :].rearrange("p c q -> p (c q)")

            # ---------------- projections ----------------------------------
            with tc.tile_pool(name="ps1", bufs=1, space="PSUM") as ps1, \
                 tc.tile_pool(name="pstp", bufs=4, space="PSUM") as pstp:
                psA = ps1.tile([128, 512], F32, tag="psA")
                psB = ps1.tile([128, 256], F32, tag="psB")
                psQA = ps1.tile([128, C, QL], F32, tag="psQA")
                psQB = ps1.tile([34, C, QL], F32, tag="psQB")

                # hat_q (DIRECTLY transposed: out[d,(c,q)] = sum_k W[k,cD+d] q[q,k])
                # shares one bank across classes; start=True clears the whole
                # bank's has_written bits, so each (c, piece) group runs its
                # start..stop back-to-back. k is split in halves combined via
                # SBUF adds (W streams in k-major pieces of 2 chunks each).
                def q_cs(h, cs):
                    ks = range(3 * h, 3 * h + 3)
                    add_bias = with_bias and h == 1
                    for c in cs:
                        for j, k in enumerate(ks):
                            nc.tensor.matmul(psQA[:, c, :], W_sb[:, k, D * c:D * c + 128],
                                             qT_sb[:, k, :], start=(j == 0),
                                             stop=(j == 2 and not add_bias))
                        if add_bias:
                            nc.tensor.matmul(psQA[:, c, :], b_sb[:, D * c:D * c + 128],
                                             onesq[:], start=False, stop=True)
                        for j, k in enumerate(ks):
                            nc.tensor.matmul(psQB[0:25, c, :], W_sb[:, k, D * c + 128:D * (c + 1)],
                                             qT_sb[:, k, :], start=(j == 0),
                                             stop=(j == 2 and not add_bias))
                        if add_bias:
                            nc.tensor.matmul(psQB[0:25, c, :], b_sb[:, D * c + 128:D * (c + 1)],
                                             onesq[:], start=False, stop=True)

                def q_copy(cs):
                    c0, c1 = cs[0], cs[-1] + 1
                    nc.vector.tensor_copy(tqA[:, c0:c1, :].rearrange("p c q -> p (c q)"),
                                          psQA[:, c0:c1, :].rearrange("p c q -> p (c q)"))
                    nc.vector.tensor_copy(tqB[0:25, c0:c1, :].rearrange("p c q -> p (c q)"),
                                          psQB[0:25, c0:c1, :].rearrange("p c q -> p (c q)"))

                def q_add(cs):
                    c0, c1 = cs[0], cs[-1] + 1
                    nc.vector.tensor_tensor(
                        tqA[:, c0:c1, :].rearrange("p c q -> p (c q)"),
                        tqA[:, c0:c1, :].rearrange("p c q -> p (c q)"),
                        psQA[:, c0:c1, :].rearrange("p c q -> p (c q)"), op=ADD)
                    nc.vector.tensor_tensor(
                        tqB[0:25, c0:c1, :].rearrange("p c q -> p (c q)"),
                        tqB[0:25, c0:c1, :].rearrange("p c q -> p (c q)"),
                        psQB[0:25, c0:c1, :].rearrange("p c q -> p (c q)"), op=ADD)

                def tposes(cs):
                    for c in cs:
                        t1 = pstp.tile([128, 128], RDT, tag="tp")
                        nc.tensor.transpose(t1[:], hat_m_bf[:, D * c:D * c + 128], eyeb[:])
                        (nc.vector.tensor_copy if c % 2 else nc.scalar.copy)(mTc1[:, c, :], t1[:])
                        t2 = pstp.tile([25, 128], RDT, tag="tp")
                        nc.tensor.transpose(t2[:], hat_m_bf[:, D * c + 128:D * (c + 1)],
                                            eyeb[:])
                        (nc.scalar.copy if c % 2 else nc.vector.tensor_copy)(mTc2[0:25, c, :], t2[:])

                # ---- hat_m accumulation, interleaved with W piece arrival ----
                # (pieces of 2 k-chunks; psA = cols 0:512, psB = 512:768)
                for k in range(KC):
                    nc.tensor.matmul(psA[:], mT_sb[:, k, :], W_sb[:, k, 0:512],
                                     start=(k == 0), stop=(k == KC - 1 and not with_bias))
                    nc.tensor.matmul(psB[:], mT_sb[:, k, :], W_sb[:, k, 512:768],
                                     start=(k == 0), stop=(k == KC - 1 and not with_bias))
                    if k == 3:
                        # k0-2 landed: first q half runs while piece 3 streams
                        q_cs(0, [0, 1, 2, 3, 4])
                        q_copy([0, 1, 2, 3, 4])
                if with_bias:
                    nc.tensor.matmul(psA[:], ones1[:], b_sb[:, 0:512], start=False, stop=True)
                    nc.tensor.matmul(psB[:, 0:253], ones1[:], b_sb[:, 512:765],
                                     start=False, stop=True)
                # hat_m straight to bf16 (transposes/stats read bf16; the
                # f32 staging tile and its extra copies are gone)
                nc.scalar.copy(hat_m_bf[:, 0:256], psA[:, 0:256])
                nc.vector.tensor_copy(hat_m_bf[:, 256:512], psA[:, 256:512])
                nc.scalar.copy(hat_m_bf[:, 512:640], psB[:, 0:128])
                nc.vector.tensor_copy(hat_m_bf[:, 640:765], psB[:, 128:253])
                q_cs(1, [0, 1, 2, 3, 4])
                q_add([0, 1, 2, 3, 4])

                # ---- pearson #1 tq-side (PE mms issued before the transposes
                # so the yn1 row chain finishes well before num1 needs it) ----
                sqA1 = sb2.tile([128, CQ], RDT, tag="sqA")
                nc.gpsimd.tensor_tensor(sqA1[:], tqAf, tqAf, op=MUL)
                sqB1 = sb2.tile([25, CQ], RDT, tag="sqB")
                nc.vector.tensor_tensor(sqB1[:], tqBf25, tqBf25, op=MUL)
                colsum1 = ps1.tile([1, CQ], F32, tag="psQA")
                nc.tensor.matmul(colsum1[:], onesF[:], tqAf, start=True, stop=False)
                nc.tensor.matmul(colsum1[:], onesF[0:25], tqBf25, start=False, stop=True)
                nc.scalar.copy(tqB32f[:], colsum1[:])
                csr_t = sb2.tile([1, CQ], F32, tag="csr")
                nc.vector.tensor_copy(csr_t[:], colsum1[:])
                yn2r1 = ps1.tile([1, CQ], F32, tag="psQB")
                nc.tensor.matmul(yn2r1[:], onesF[:], sqA1[:], start=True, stop=False)
                nc.tensor.matmul(yn2r1[:], onesF[0:25], sqB1[:], start=False, stop=True)
                csqv1 = sb2.tile([1, CQ], F32, tag="csqv")
                nc.vector.tensor_tensor(csqv1[:], tqB32f, tqB32f, op=MUL)
                yn2_t = sb2.tile([1, CQ], F32, tag="yn2")
                nc.vector.scalar_tensor_tensor(yn2_t[:], csqv1[:], -1.0 / D, yn2r1[:],
                                               op0=MUL, op1=ADD)



                tposes([0, 1, 2])
                tposes([3, 4])

                # mTc2 row 32 = -mean_d(hat_m)[c,i] via ones-matmul over mTc
                # (split 512+128: PSUM banks hold 512 fp32/partition max;
                #  reuses the psA/psB banks, free after the hat_m copies)
                psMuA = ps1.tile([1, 512], F32, tag="psA")
                psMuB = ps1.tile([1, 128], F32, tag="psB")
                mTc1f = mTc1[:].rearrange("p c n -> p (c n)")
                mTc2f = mTc2[0:25].rearrange("p c n -> p (c n)")
                nc.tensor.matmul(psMuA[:], onesF[:], mTc1f[:, 0:512],
                                 start=True, stop=False)
                nc.tensor.matmul(psMuA[:], onesF[0:25], mTc2f[:, 0:512],
                                 start=False, stop=True)
                nc.tensor.matmul(psMuB[:], onesF[:], mTc1f[:, 512:640],
                                 start=True, stop=False)
                nc.tensor.matmul(psMuB[:], onesF[0:25], mTc2f[:, 512:640],
                                 start=False, stop=True)
                mTc2r32 = mTc2[32:33, :, :].rearrange("p c n -> p (c n)")
                nc.scalar.activation(mTc2r32[:, 0:512], psMuA[:], ACT.Copy, scale=-1.0 / D)
                nc.scalar.activation(mTc2r32[:, 512:640], psMuB[:], ACT.Copy, scale=-1.0 / D)

                # ---- inv_xn [128, C] (cheap engines, off the p1 chain) --------
                # squares straight from PSUM (before hat_m copies land), then
                # two DVE reduces; xn2 = sum hm^2 - (sum hm)^2/D.
                sqall = sb2.tile([128, CD], F32, tag="sqall")
                nc.scalar.activation(sqall[:, 0:512], psA[:], ACT.Square)
                nc.scalar.activation(sqall[:, 512:765], psB[:, 0:253], ACT.Square)
                xn2r = sb.tile([128, C], F32, tag="xn2r")
                nc.vector.tensor_reduce(xn2r[:],
                                        sqall[:].rearrange("p (c d) -> p c d", c=C),
                                        axis=AX, op=ADD)
                csum = sb.tile([128, C], F32, tag="csum")
                nc.vector.tensor_reduce(csum[:],
                                        hat_m_bf[:, 0:CD].rearrange("p (c d) -> p c d", c=C),
                                        axis=AX, op=ADD)
                csum2 = sb.tile([128, C], F32, tag="csum2")
                nc.vector.scalar_tensor_tensor(csum2[:], csum[:], 1.0 / D, csum[:],
                                               op0=MUL, op1=MUL)
                xn2 = sb.tile([128, C], F32, tag="xn2")
                nc.vector.tensor_tensor(xn2[:], xn2r[:], csum2[:], op=SUB)
                lxn = sb.tile([128, C], F32, tag="lxn")
                nc.scalar.activation(lxn[:], xn2[:], ACT.Ln)
                inv_xn = sb.tile([128, C], F32, tag="invxn")
                nc.scalar.activation(inv_xn[:], lxn[:], ACT.Exp, scale=-0.5)


            if dbg:
                nc.sync.dma_start(dbg_d["hatm"][:], hat_m_bf[:, 0:CD])
                nc.sync.dma_start(dbg_d["tqA0"][:], tqAf)
                nc.sync.dma_start(dbg_d["tqB0"][:], tqB[:].rearrange("p c q -> p (c q)"))
                nc.sync.dma_start(dbg_d["mTc1d"][:], mTc1[:].rearrange("p c n -> p (c n)"))
                nc.sync.dma_start(dbg_d["mTc2d"][:], mTc2[:].rearrange("p c n -> p (c n)"))

            ixb = inv_xn[:].rearrange("p (c a) -> p c a", a=1).broadcast_to((128, C, QL))

            # ---------------- routing --------------------------------------
            # Recursions keep tq-dependent matmuls off the critical path:
            #   u_{t+1}    = u_t + s_eff*hv   (s_eff = 2^t s; pearson is scale-
            #                                  invariant in u so the reference's
            #                                  0.5 shrink per step is dropped)
            #   cs_{t+1}   = cs_t + s_eff*csh             (tqB row 32)
            #   numX_{t+1} = numX_t + s_eff*(ixn*numv)    (numv = mTc^T.hv_cent)
            # yn2 = sum u^2 - cs^2/D is recomputed directly from updated u.
            # hv/fps consume softmax(d) and p as separate matmul passes
            # (hv = hm^T d + hm^T p), so dsp is never materialized.
            with tc.tile_pool(name="ps2", bufs=1, space="PSUM") as ps2:
                def pearson_tail(yn2, numX):
                    """rr = 1/(1+exp(2*pp)); p = 1-2rr computed by the caller."""
                    lyn = sb2.tile([1, CQ], F32, tag="lyn")
                    nc.scalar.activation(lyn[:], yn2[:], ACT.Ln)
                    iyr = sb2.tile([1, CQ], RDT, tag="iyr")
                    nc.scalar.activation(iyr[:], lyn[:], ACT.Exp, scale=-0.5)
                    iyb = ps2.tile([128, CQ], F32, tag="bcast")
                    nc.tensor.matmul(iyb[:], ones1[:], iyr[:], start=True, stop=True)
                    pp = sb2.tile([128, CQ], F32, tag="pp")
                    nc.vector.tensor_tensor(pp[:], numX[:], iyb[:], op=MUL)
                    e2 = sb2.tile([128, CQ], F32, tag="e2")
                    nc.scalar.activation(e2[:], pp[:], ACT.Exp, scale=2.0)
                    den = sb2.tile([128, CQ], F32, tag="den")
                    nc.vector.tensor_scalar(den[:], e2[:], 1.0, None, op0=ADD)
                    rr = sb2.tile([128, CQ], F32, tag="rr")
                    nc.vector.reciprocal(rr[:], den[:])
                    return rr

                # num1 (centered via tqB row 32); numX = ixn * num1 in SBUF
                num1 = ps2.tile([128, C, QL], F32, tag="num")
                for c in range(C):
                    nc.tensor.matmul(num1[:, c, :], mTc1[:, c, :], tqA[:, c, :],
                                     start=True, stop=False)
                    nc.tensor.matmul(num1[:, c, :], mTc2[:, c, :], tqB[:, c, :],
                                     start=False, stop=True)
                numX_t = sb2.tile([128, CQ], F32, tag="numX")
                nc.vector.tensor_tensor(numX_t[:].rearrange("p (c q) -> p c q", c=C),
                                        num1[:], ixb, op=MUL)
                rr_t = pearson_tail(yn2_t, numX_t)

                a_t = None
                dd_t = None

                for it in range(2):
                    # p = 1 - 2rr (bf16: PE moving operand + w factor)
                    p_t = sb2.tile([128, C, QL], RDT, tag="p")
                    p_f = p_t[:].rearrange("p c q -> p (c q)")
                    nc.vector.tensor_scalar(p_f, rr_t[:], -2.0, 1.0, op0=MUL, op1=ADD)
                    if dbg and it == 1:
                        nc.sync.dma_start(dbg_d["p2"][:], p_f)

                    # hv = hm^T.(d + p): two accumulating matmul passes
                    ddsrc = (lambda c: c02_t[:]) if it == 0 else (lambda c: dd_t[:, c, :])
                    hvA = ps2.tile([128, C, QL], F32, tag="hvA")
                    hvB = ps2.tile([26, C, QL], F32, tag="hvB")
                    for c in range(C):
                        nc.tensor.matmul(hvA[:, c, :], hat_m_bf[:, D * c:D * c + 128],
                                         ddsrc(c), start=True, stop=False)
                        nc.tensor.matmul(hvA[:, c, :], hat_m_bf[:, D * c:D * c + 128],
                                         p_t[:, c, :], start=False, stop=True)
                        nc.tensor.matmul(hvB[:, c, :], hat_m_bf[:, D * c + 128:D * c + 154],
                                         ddsrc(c), start=True, stop=False)
                        nc.tensor.matmul(hvB[:, c, :], hat_m_bf[:, D * c + 128:D * c + 154],
                                         p_t[:, c, :], start=False, stop=True)
                    hvAf = hvA[:].rearrange("p c q -> p (c q)")
                    hvBf25 = hvB[0:25].rearrange("p c q -> p (c q)")

                    # stage hv; squares for n2 (sqhA first: it gates n2->squash)
                    sqhA = sb2.tile([128, CQ], RDT, tag="sqhA")
                    nc.scalar.activation(sqhA[:], hvAf, ACT.Square)
                    nc.vector.tensor_copy(vBf25[:], hvBf25)
                    nc.scalar.copy(vAf[:], hvAf)
                    sqhB = sb2.tile([25, CQ], RDT, tag="sqhB")
                    nc.vector.tensor_tensor(sqhB[:], vBf25[:], vBf25[:], op=MUL)
                    csh = ps2.tile([1, CQ], F32, tag="colsum")
                    nc.tensor.matmul(csh[:], onesF[:], vAf[:], start=True, stop=False)
                    nc.tensor.matmul(csh[:], onesF[0:25], vBf25[:], start=False, stop=True)
                    nc.scalar.copy(vB32f[:], csh[:])
                    n2 = ps2.tile([1, CQ], F32, tag="cs2")
                    nc.tensor.matmul(n2[:], onesF[:], sqhA[:], start=True, stop=False)
                    nc.tensor.matmul(n2[:], onesF[0:25], sqhB[:], start=False, stop=True)

                    # numv = mTc^T.[hv; row32=csh] (early; consumed after sBh)
                    numv = ps2.tile([128, C, QL], F32, tag="num")
                    for c in range(C):
                        nc.tensor.matmul(numv[:, c, :], mTc1[:, c, :], vA[:, c, :],
                                         start=True, stop=False)
                        nc.tensor.matmul(numv[:, c, :], mTc2[:, c, :], vB34[:, c, :],
                                         start=False, stop=True)

                    # squash scale: nsrow = -2^it*s, s = sqrt(n2)/(1+n2);
                    # Act chain first so it isn't queued behind DVE siblings
                    ln2 = sb2.tile([1, CQ], F32, tag="ln2")
                    nc.scalar.activation(ln2[:], n2[:], ACT.Ln, bias=epsb[0:1, :])
                    r2 = sb2.tile([1, CQ], F32, tag="r2")
                    nc.scalar.activation(r2[:], ln2[:], ACT.Exp, scale=-0.5,
                                         bias=(ln2b[:] if it else 0.0))
                    n2p1 = sb2.tile([1, CQ], F32, tag="n2p1")
                    nc.vector.tensor_scalar(n2p1[:], n2[:], 1.0, None, op0=ADD)
                    r1 = sb2.tile([1, CQ], F32, tag="r1")
                    nc.vector.reciprocal(r1[:], n2p1[:])
                    nsrow = sb2.tile([1, CQ], RDT, tag="nsrow")
                    nc.vector.scalar_tensor_tensor(nsrow[:], r1[:], 1.0, r2[:],
                                                   op0=SUB, op1=MUL)
                    sBh = ps2.tile([128, CQ], F32, tag="bcast")
                    nc.tensor.matmul(sBh[:], mones1[:], nsrow[:], start=True, stop=True)

                    # numX += s_eff * (ixn*numv)
                    numvx = sb2.tile([128, CQ], F32, tag="numvx")
                    nc.vector.tensor_tensor(numvx[:].rearrange("p (c q) -> p c q", c=C),
                                            numv[:], ixb, op=MUL)
                    # u update + cs update
                    svA = sb2.tile([128, C, QL], RDT, tag="svA")
                    nc.vector.tensor_tensor(svA[:].rearrange("p c q -> p (c q)"),
                                            vAf[:], sBh[:], op=MUL)
                    svB = sb2.tile([25, C, QL], RDT, tag="svB")
                    nc.vector.tensor_tensor(svB[:].rearrange("p c q -> p (c q)"),
                                            vBf25[:], sBh[0:25, :], op=MUL)
                    nc.vector.tensor_tensor(tqAf, tqAf,
                                            svA[:].rearrange("p c q -> p (c q)"), op=ADD)
                    nc.vector.tensor_tensor(tqBf25, tqBf25,
                                            svB[:].rearrange("p c q -> p (c q)"), op=ADD)
                    # cs += s_eff*csh kept as a partition-0 row (csh stays in
                    # PSUM so the DVE same-base-partition rule is satisfied)
                    t3 = sb2.tile([1, CQ], F32, tag="t3")
                    nc.vector.scalar_tensor_tensor(t3[:], csh[:], -1.0, nsrow[:],
                                                   op0=MUL, op1=MUL)
                    csr_new = sb2.tile([1, CQ], F32, tag="csr")
                    nc.vector.tensor_tensor(csr_new[:], csr_t[:], t3[:], op=ADD)
                    csr_t = csr_new
                    # yn2 = sum u^2 - cs^2/D (direct, from updated u)
                    sqA = sb2.tile([128, CQ], RDT, tag="sqA")
                    nc.scalar.activation(sqA[:], tqAf, ACT.Square)
                    sqB = sb2.tile([25, CQ], RDT, tag="sqB")
                    nc.vector.tensor_tensor(sqB[:], tqBf25, tqBf25, op=MUL)
                    yn2r = ps2.tile([1, CQ], F32, tag="yn")
                    nc.tensor.matmul(yn2r[:], onesF[:], sqA[:], start=True, stop=False)
                    nc.tensor.matmul(yn2r[:], onesF[0:25], sqB[:], start=False, stop=True)
                    csqv = sb2.tile([1, CQ], F32, tag="csqv")
                    nc.vector.tensor_tensor(csqv[:], csr_t[:], csr_t[:], op=MUL)
                    yn2_new = sb2.tile([1, CQ], F32, tag="yn2")
                    nc.vector.scalar_tensor_tensor(yn2_new[:], csqv[:], -1.0 / D,
                                                   yn2r[:], op0=MUL, op1=ADD)
                    yn2_t = yn2_new

                    # mdv on scaled sv => a += p * (0.5^it) * mdv'
                    mdv = ps2.tile([128, C, QL], F32, tag="mdv")
                    for c in range(C):
                        nc.tensor.matmul(mdv[:, c, :], mTc1[:, c, :], svA[:, c, :],
                                         start=True, stop=False)
                        nc.tensor.matmul(mdv[:, c, :], mTc2[0:25, c, :], svB[:, c, :],
                                         start=False, stop=True)
                    nv2 = sb2.tile([128, CQ], F32, tag="nv2")
                    nc.vector.tensor_tensor(nv2[:], numvx[:], sBh[:], op=MUL)
                    numX_new = sb2.tile([128, CQ], F32, tag="numX")
                    nc.gpsimd.tensor_tensor(numX_new[:], numX_t[:], nv2[:], op=ADD)
                    numX_t = numX_new
                    w = sb2.tile([128, CQ], F32, tag="w")
                    nc.vector.scalar_tensor_tensor(w[:],
                                                   mdv[:].rearrange("p c q -> p (c q)"),
                                                   0.5 ** it, p_f, op0=MUL, op1=MUL)
                    if it == 0:
                        a_t = w
                    else:
                        a_new = sb2.tile([128, CQ], F32, tag="a")
                        nc.gpsimd.tensor_tensor(a_new[:], a_t[:], w[:], op=ADD)
                        a_t = a_new
                    if dbg:
                        nc.sync.dma_start(dbg_d["a1" if it == 0 else "a2"][:], a_t[:])

                    pp_rr = pearson_tail(yn2_t, numX_t)

                    # softmax of a_t (overlaps the pearson tail; dd feeds the
                    # next hv / final fps matmuls directly)
                    ea = sb2.tile([128, CQ], F32, tag="ea")
                    nc.scalar.activation(ea[:], a_t[:], ACT.Exp)
                    asum = sb2.tile([128, QL], F32, tag="asum")
                    nc.vector.tensor_reduce(asum[:],
                                            ea[:].rearrange("p (c q) -> p q c", c=C),
                                            axis=AX, op=ADD)
                    rs = sb2.tile([128, QL], F32, tag="rs")
                    nc.vector.reciprocal(rs[:], asum[:])
                    dd_t = sb2.tile([128, C, QL], RDT, tag="dd")
                    nc.gpsimd.tensor_tensor(
                        dd_t[:], ea[:].rearrange("p (c q) -> p c q", c=C),
                        rs[:].rearrange("p (a q) -> p a q", a=1).broadcast_to((128, C, QL)),
                        op=MUL)
                    rr_t = pp_rr

                # ---------------- final ------------------------------------
                pF = sb2.tile([128, C, QL], RDT, tag="p")
                nc.vector.tensor_scalar(pF[:].rearrange("p c q -> p (c q)"),
                                        rr_t[:], -2.0, 1.0, op0=MUL, op1=ADD)

                # final fps = (d + p)^T.hm per class, N=256 window (cols 765+
                # zero); each class in its own PSUM bank.
                n2q = sb2.tile([QL, C], F32, tag="n2q")
                fps_l = []
                for c, tg in zip(range(C), ("hvA", "hvB", "num", "mdv", "colsum")):
                    fps = ps2.tile([QL, 256], F32, tag=tg)
                    nc.tensor.matmul(fps[:], dd_t[:, c, :], hat_m_bf[:, D * c:D * c + 256],
                                     start=True, stop=False)
                    nc.tensor.matmul(fps[:], pF[:, c, :], hat_m_bf[:, D * c:D * c + 256],
                                     start=False, stop=True)
                    fps_l.append(fps)
                    if c < 3:
                        sqf = sb2.tile([QL, D], F32, tag="sqf")
                        nc.scalar.activation(sqf[:], fps[:, 0:D], ACT.Square)
                        jnk = sb2.tile([QL, D], F32, tag="jnk")
                        nc.vector.tensor_scalar(jnk[:], sqf[:], 1.0, 0.0, op0=MUL, op1=ADD,
                                                accum_out=n2q[:, c:c + 1])
                    else:
                        hvFc = sb2.tile([QL, D], F32, tag="hvFc")
                        nc.vector.tensor_copy(hvFc[:], fps[:, 0:D])
                        sqf = sb2.tile([QL, D], F32, tag="sqf")
                        nc.vector.scalar_tensor_tensor(sqf[:], hvFc[:], 1.0, hvFc[:],
                                                       op0=MUL, op1=MUL,
                                                       accum_out=n2q[:, c:c + 1])
                # fs = (1 - 1/(1+n2q)) / sqrt(n2q+eps), two column groups
                fp1 = sb2.tile([QL, C], F32, tag="fp1")
                fr1 = sb2.tile([QL, C], F32, tag="fr1")
                fln = sb2.tile([QL, C], F32, tag="fln")
                fr2 = sb2.tile([QL, C], F32, tag="fr2")
                omr = sb2.tile([QL, C], F32, tag="omr")
                fs = sb2.tile([QL, C], F32, tag="fs")
                outT = sb.tile([QL, CD], F32, tag="outT")
                for c0, c1 in ((0, 3), (3, 5)):
                    nc.vector.tensor_scalar(fp1[:, c0:c1], n2q[:, c0:c1], 1.0, None, op0=ADD)
                    nc.vector.reciprocal(fr1[:, c0:c1], fp1[:, c0:c1])
                    nc.scalar.activation(fln[:, c0:c1], n2q[:, c0:c1], ACT.Ln,
                                         bias=epsb[0:QL, :])
                    nc.scalar.activation(fr2[:, c0:c1], fln[:, c0:c1], ACT.Exp, scale=-0.5)
                    nc.vector.tensor_scalar(omr[:, c0:c1], fr1[:, c0:c1], -1.0, 1.0,
                                            op0=MUL, op1=ADD)
                    nc.vector.tensor_tensor(fs[:, c0:c1], omr[:, c0:c1], fr2[:, c0:c1], op=MUL)
                    for c in range(c0, c1):
                        if c % 2 == 0:
                            nc.vector.tensor_scalar(outT[:, D * c:D * (c + 1)],
                                                    fps_l[c][:, 0:D],
                                                    fs[:, c:c + 1], None, op0=MUL)
                        else:
                            nc.scalar.activation(outT[:, D * c:D * (c + 1)],
                                                 fps_l[c][:, 0:D], ACT.Copy,
                                                 scale=fs[:, c:c + 1])
                nc.sync.dma_start(out_d[:], outT[:])

    # All activations use only {Ln, Exp, Copy, Square}, which live together in
    # act func set 6 (natural_log_exp_and_others). The default solver alternates
    # sets, inserting table reloads (~1.3us each); one load suffices.
    def _single_act_table_load():
        inst = mybir.InstLoadActFuncSet(
            name=nc.get_next_instruction_name(), ins=[], outs=[],
            act_func_set_id=6,
        )
        inst.engine = mybir.EngineType.Activation
        nc.register_instruction(inst)
        for blk in nc.main_func.blocks:
            for idx, bi in enumerate(blk.instructions):
                if isinstance(bi, mybir.InstActivation):
                    blk.instructions.insert(idx, inst)
                    return
        raise AssertionError("no activation found")

    nc.insert_act_table_loads = _single_act_table_load
    nc.compile()
    return nc


_CACHE = {}
LAST_EXEC_NS = None
LAST_RESULTS = None


def host_inputs(m, q_slice, W, b, with_bias):
    """Per-core input map (layout/dtype transforms only) for one 32-query slice.

    qm[p, :] = [qT k-major | mT k-major]; W k-major [128, 6*768]. Each partition
    row is one contiguous run so DMAs avoid the small-element latency penalty.
    """
    if str(WDT).endswith("bfloat16"):
        import ml_dtypes
        wnp = ml_dtypes.bfloat16
    else:
        wnp = np.float32
    Wp = np.zeros((K, NPAD), dtype=np.float32)
    Wp[:, :CD] = W
    # [768, 768] -> [128 p, 6 k, 768 n] -> [128, 4608]
    Wk = np.ascontiguousarray(
        Wp.reshape(KC, 128, NPAD).transpose(1, 0, 2).reshape(128, KC * NPAD)
    ).astype(wnp)
    mTk = m.T.reshape(KC, 128, I).transpose(1, 0, 2).reshape(128, KC * I)
    qTk = q_slice.T.reshape(KC, 128, QL).transpose(1, 0, 2).reshape(128, KC * QL)
    qm = np.ascontiguousarray(np.concatenate([qTk, mTk], axis=1)).astype(wnp)
    im = {"qm": qm, "Wp": Wk}
    if with_bias:
        im["b"] = b.reshape(1, CD)
    return im


def kernel(m, q, W, b):
    m = np.asarray(m, dtype=np.float32)
    q = np.asarray(q, dtype=np.float32)
    W = np.asarray(W, dtype=np.float32)
    b = np.asarray(b, dtype=np.float32)
    assert m.shape == (I, K) and q.shape == (NCORES * QL, K) and W.shape == (K, CD)

    with_bias = bool(np.any(b))
    dbg = bool(int(os.environ.get("KERNEL_DBG", "0")))
    key = ("v2", with_bias, str(DT), dbg)
    if key not in _CACHE:
        _CACHE[key] = build(with_bias, dbg)
    nc = _CACHE[key]

    in_maps = [host_inputs(m, q[QL * i:QL * (i + 1)], W, b, with_bias)
               for i in range(NCORES)]

    res = run_bass_kernel_spmd(nc, in_maps, list(range(NCORES)))
    global LAST_EXEC_NS, LAST_RESULTS
    LAST_EXEC_NS = res.exec_time_ns
    LAST_RESULTS = res.results
    out = np.concatenate([res.results[i]["out"] for i in range(NCORES)], axis=0)
    return out.astype(np.float32)


if __name__ == "__main__":
    rng = np.random.default_rng(0)
    m = rng.standard_normal((I, K)).astype(np.float32)
    q = rng.standard_normal((NCORES * QL, K)).astype(np.float32)
    W = (rng.standard_normal((K, CD)) * 0.02).astype(np.float32)
    b = np.zeros((CD,), dtype=np.float32)
    out = kernel(m=m, q=q, W=W, b=b)
    print("out", out.shape, out.dtype, np.abs(out).mean())

